# revision 9
# baseline (speedup 1.0000x reference)
"""Trainium2 Bass kernel for nn_Attention_66640712565145.

Attention with external K and full [h,n,n] bias:
  qv = x @ w_qv ; q,v = split(qv)
  dots = (q @ ext_k^T + ext_bias) * scale ; attn = softmax(dots)
  out = (attn @ v) @ w_out + b_out

Sharding: data-parallel over batch across 8 cores (2 batches/core), no
collectives. Per core the kernel computes S transposed (S^T[m,n], partition =
keys) so the PV matmul needs no transposes; softmax row-sums come from a ones
column embedded in the V tile layout; normalization is applied late to the
small attention output via batched reciprocal + K=1 broadcast matmuls.

Device dtypes: fp32r (TF32-like, full-speed matmul) for all matmuls, bias
pre-scaled and shipped as bf16 (added into PSUM via an identity matmul), fp32
accumulation everywhere.
"""
import numpy as np
import ml_dtypes

HEADS = 12
DIM_HEAD = 64
SCALE = DIM_HEAD ** -0.5
B, N, DIM = 16, 1024, 768
CORES = 8
BPC = B // CORES          # batches per core
T = BPC * N               # tokens per core
NT = N // 128             # 128-row tiles per sequence
VW = DIM_HEAD + 1         # v block width incl. ones column

_program = None
DEBUG = False


def _build_program():
    import concourse.bacc as bacc
    import concourse.mybir as mybir
    from concourse.tile import TileContext
    from concourse.masks import make_identity

    dt = mybir.dt
    f32, f32r, bf16 = dt.float32, dt.float32r, dt.bfloat16
    EXP = mybir.ActivationFunctionType.Exp

    nc = bacc.Bacc(None, target_bir_lowering=False)

    a_xT = nc.declare_dram_parameter("xT", [BPC, DIM, N], f32r, isOutput=False)
    a_wq = nc.declare_dram_parameter("wq", [DIM, DIM], f32r, isOutput=False)
    a_wv = nc.declare_dram_parameter("wv", [DIM, DIM], f32r, isOutput=False)
    a_kT = nc.declare_dram_parameter("kT", [HEADS // 2, 128, N], f32r, isOutput=False)
    a_bias = nc.declare_dram_parameter("biasT", [HEADS, N, N], bf16, isOutput=False)
    a_wo = nc.declare_dram_parameter("wo", [DIM, DIM], f32r, isOutput=False)
    a_out = nc.declare_dram_parameter("out", [T, DIM], f32, isOutput=True)
    qT_dram = nc.dram_tensor("qT_scratch", [BPC, DIM, N], f32r)

    ET = DIM // 128        # e tiles (6)
    if DEBUG:
        d_q = nc.declare_dram_parameter("dbg_q", [BPC, DIM, N], f32, isOutput=True)
        d_v = nc.declare_dram_parameter("dbg_v", [128, BPC * NT * HEADS * VW], f32, isOutput=True)
        d_l = nc.declare_dram_parameter("dbg_l", [128, (HEADS // 2) * N], f32, isOutput=True)
        d_a = nc.declare_dram_parameter("dbg_a", [128, ET * T], f32, isOutput=True)

    with TileContext(nc) as tc:
        with tc.tile_pool(name="persist", bufs=1) as pers:
            ident = pers.tile([128, 128], bf16, name="ident")
            make_identity(nc, ident)
            ones_f = pers.tile([128, HEADS], f32, name="ones_f")
            nc.vector.memset(ones_f, 1.0)
            ones_r = pers.tile([128, 64], f32r, name="ones_r")
            of64 = pers.tile([128, 64], f32, name="of64")
            nc.vector.memset(of64, 1.0)
            with nc.allow_low_precision("exact const"):
                nc.vector.tensor_copy(ones_r, of64)

            # long-lived buffers
            v_sb = pers.tile([128, BPC * NT * HEADS * VW], f32r, name="v_sb")
            a_sb = pers.tile([128, ET * T], f32r, name="a_sb")
            l_sb = pers.tile([128, (HEADS // 2) * N], f32r, name="l_sb")

            # ---------------- phase 1: QV projections ----------------
            with tc.tile_pool(name="ph1_w", bufs=1) as wpool, \
                 tc.tile_pool(name="ph1_x", bufs=1) as xpool, \
                 tc.tile_pool(name="ph1_s", bufs=3) as spool, \
                 tc.tile_pool(name="ph1_p", bufs=2, space="PSUM") as ppool:
                wq_t, wv_t = [], []
                for ct in range(ET):
                    t1 = wpool.tile([128, DIM], f32r, name=f"wq{ct}")
                    nc.sync.dma_start(out=t1, in_=a_wq[ct * 128:(ct + 1) * 128, :])
                    wq_t.append(t1)
                    t2 = wpool.tile([128, DIM], f32r, name=f"wv{ct}")
                    nc.sync.dma_start(out=t2, in_=a_wv[ct * 128:(ct + 1) * 128, :])
                    wv_t.append(t2)
                for b in range(BPC):
                    xT_t = []
                    for ct in range(ET):
                        t3 = xpool.tile([128, N], f32r, name=f"xT{ct}")
                        nc.sync.dma_start(out=t3, in_=a_xT[b, ct * 128:(ct + 1) * 128, :])
                        xT_t.append(t3)
                    # q^T: psum[e1, n] = sum_c wq[c, e1].T @ xT[c, n]
                    for et in range(ET):
                        pq = ppool.tile([128, N], f32, name="pq")
                        for ct in range(ET):
                            for nch in range(2):
                                nsl = slice(nch * 512, (nch + 1) * 512)
                                nc.tensor.matmul(
                                    pq[:, nsl],
                                    wq_t[ct][:, et * 128:(et + 1) * 128],
                                    xT_t[ct][:, nsl],
                                    start=(ct == 0), stop=(ct == ET - 1))
                        sq = spool.tile([128, N], f32r, name="sq")
                        with nc.allow_low_precision("fp32r matmul feed"):
                            nc.vector.tensor_copy(sq, pq)
                        nc.sync.dma_start(out=qT_dram[b, et * 128:(et + 1) * 128, :],
                                          in_=sq)
                    # v: psum[t1, e] = sum_c xT[c, t1].T @ wv[c, e]
                    for tt in range(NT):
                        pv = ppool.tile([128, DIM], f32, name="pv")
                        for ct in range(ET):
                            for osl in (slice(0, 512), slice(512, DIM)):
                                nc.tensor.matmul(
                                    pv[:, osl],
                                    xT_t[ct][:, tt * 128:(tt + 1) * 128],
                                    wv_t[ct][:, osl],
                                    start=(ct == 0), stop=(ct == ET - 1))
                        base = (b * NT + tt) * HEADS * VW
                        dst = v_sb[:, base:base + HEADS * VW]
                        dst3 = dst.rearrange("p (h w) -> p h w", w=VW)
                        with nc.allow_low_precision("fp32r matmul feed"):
                            nc.vector.tensor_copy(
                                dst3[:, :, 0:DIM_HEAD],
                                pv.rearrange("p (h w) -> p h w", w=DIM_HEAD))
                            nc.vector.tensor_copy(
                                dst3[:, :, DIM_HEAD:VW].rearrange("p h w -> p (h w)"),
                                ones_f)

            # ---------------- phase 2: attention sweeps ----------------
            with tc.tile_pool(name="ph2_bias", bufs=1) as bpool, \
                 tc.tile_pool(name="ph2_q", bufs=2) as qpool, \
                 tc.tile_pool(name="ph2_pt", bufs=3) as ptpool, \
                 tc.tile_pool(name="ph2_s", bufs=2, space="PSUM") as ps_s, \
                 tc.tile_pool(name="ph2_o", bufs=1, space="PSUM") as ps_o:
                kT_sb = bpool.tile([128, (HEADS // 2) * N], f32r, name="kT_sb")
                for hp in range(HEADS // 2):
                    nc.sync.dma_start(out=kT_sb[:, hp * N:(hp + 1) * N],
                                      in_=a_kT[hp, :, :])
                for hp in range(HEADS // 2):
                    bias_t = []
                    for h2 in range(2):
                        h = 2 * hp + h2
                        tb = bpool.tile([128, NT * N], bf16, name=f"bias{h2}")
                        nc.sync.dma_start(
                            out=tb.rearrange("p (t n) -> p t n", n=N),
                            in_=a_bias[h, :, :].rearrange("(t p) n -> p t n", p=128))
                        bias_t.append(tb)
                    for b in range(BPC):
                        tq = qpool.tile([128, N], f32r, name="tq")
                        nc.sync.dma_start(out=tq,
                                          in_=qT_dram[b, hp * 128:(hp + 1) * 128, :])
                        po = [ps_o.tile([VW, N], f32, name=f"po{h2}")
                              for h2 in range(2)]
                        for mt in range(NT):
                            pss = [ps_s.tile([128, N], f32, name="ps")
                                   for _ in range(2)]
                            for h2 in range(2):
                                r0 = h2 * 64
                                for nch in range(2):
                                    nsl = slice(nch * 512, (nch + 1) * 512)
                                    nc.tensor.matmul(
                                        pss[h2][:, nsl],
                                        kT_sb[r0:r0 + 64,
                                              hp * N + mt * 128:hp * N + (mt + 1) * 128],
                                        tq[r0:r0 + 64, nsl],
                                        start=True, stop=False,
                                        tile_position=(r0, 0))
                                for nch in range(2):
                                    nsl = slice(nch * 512, (nch + 1) * 512)
                                    nc.tensor.matmul(
                                        pss[h2][:, nsl], ident,
                                        bias_t[h2][:, mt * N:(mt + 1) * N][:, nsl],
                                        start=False, stop=True)
                            for h2 in range(2):
                                h = 2 * hp + h2
                                pt = ptpool.tile([128, N], f32r, name="pt")
                                nc.scalar.activation(pt, pss[h2], EXP)
                                vbase = (b * NT + mt) * HEADS * VW + h * VW
                                vsl = v_sb[:, vbase:vbase + VW]
                                for nch in range(2):
                                    nsl = slice(nch * 512, (nch + 1) * 512)
                                    nc.tensor.matmul(
                                        po[h2][:, nsl], vsl, pt[:, nsl],
                                        start=(mt == 0), stop=(mt == NT - 1))
                        for h2 in range(2):
                            u = hp * 4 + b * 2 + h2          # unit index 0..23
                            with nc.allow_low_precision("fp32r matmul feed"):
                                nc.vector.tensor_copy(
                                    a_sb[h2 * 64:(h2 + 1) * 64,
                                         hp * T + b * N: hp * T + (b + 1) * N],
                                    po[h2][0:DIM_HEAD, :])
                                nc.vector.tensor_copy(
                                    l_sb[(u % 4) * 32:(u % 4) * 32 + 1,
                                         (u // 4) * N:(u // 4 + 1) * N],
                                    po[h2][DIM_HEAD:VW, :])

            # ---------------- phase 3: normalize + output projection ----
            with tc.tile_pool(name="ph3_w", bufs=1) as wopool, \
                 tc.tile_pool(name="ph3_s", bufs=3) as s3pool, \
                 tc.tile_pool(name="ph3_p", bufs=2, space="PSUM") as p3pool, \
                 tc.tile_pool(name="ph3_pb", bufs=2, space="PSUM") as pbpool:
                with nc.allow_low_precision("softmax reciprocal"):
                    nc.vector.reciprocal(l_sb, l_sb)
                for u in range(4 * (HEADS // 2)):
                    hp, b, h2 = u // 4, (u % 4) // 2, u % 2
                    row = (u % 4) * 32
                    pb = pbpool.tile([64, N], f32, name="pb")
                    for nch in range(2):
                        nsl = slice(nch * 512, (nch + 1) * 512)
                        nc.tensor.matmul(
                            pb[:, nsl], ones_r[row:row + 1, :],
                            l_sb[row:row + 1, (u // 4) * N:(u // 4) * N + N][:, nsl],
                            start=True, stop=True, tile_position=(row, 0))
                    asl = a_sb[h2 * 64:(h2 + 1) * 64,
                               hp * T + b * N: hp * T + (b + 1) * N]
                    with nc.allow_low_precision("normalize"):
                        nc.vector.tensor_mul(asl, asl, pb)

                wo_t = []
                for et in range(ET):
                    t4 = wopool.tile([128, DIM], f32r, name=f"wo{et}")
                    nc.sync.dma_start(out=t4, in_=a_wo[et * 128:(et + 1) * 128, :])
                    wo_t.append(t4)
                for tt in range(T // 128):
                    pp = p3pool.tile([128, DIM], f32, name="pp")
                    for et in range(ET):
                        for osl in (slice(0, 512), slice(512, DIM)):
                            nc.tensor.matmul(
                                pp[:, osl],
                                a_sb[:, et * T + tt * 128: et * T + (tt + 1) * 128],
                                wo_t[et][:, osl],
                                start=(et == 0), stop=(et == ET - 1))
                    so = s3pool.tile([128, DIM], f32, name="so")
                    nc.vector.tensor_copy(so, pp)
                    nc.sync.dma_start(out=a_out[tt * 128:(tt + 1) * 128, :], in_=so)
                if DEBUG:
                    nc.sync.dma_start(out=d_q[:, :, :], in_=qT_dram[:, :, :].bitcast(f32))
                    nc.sync.dma_start(out=d_v[:, :], in_=v_sb.bitcast(f32))
                    nc.sync.dma_start(out=d_l[:, :], in_=l_sb.bitcast(f32))
                    nc.sync.dma_start(out=d_a[:, :], in_=a_sb.bitcast(f32))

    nc.finalize()
    return nc


def _get_program():
    global _program
    if _program is None:
        _program = _build_program()
    return _program


def kernel(x, w_qv, ext_k, ext_bias, w_out, b_out):
    from concourse.bass_utils import run_bass_kernel_spmd

    nc = _get_program()

    x = np.asarray(x, dtype=np.float32)
    w_qv = np.asarray(w_qv, dtype=np.float32)
    ext_k = np.asarray(ext_k, dtype=np.float32)
    ext_bias = np.asarray(ext_bias, dtype=np.float32)
    w_out = np.asarray(w_out, dtype=np.float32)
    b_out = np.asarray(b_out, dtype=np.float32)

    w_q = np.ascontiguousarray(w_qv[:, :DIM] * SCALE)
    w_v = np.ascontiguousarray(w_qv[:, DIM:])
    # kT packed head pairs: [6, 128, N]; rows 0:64 head 2hp, 64:128 head 2hp+1
    k0 = ext_k[0]                                    # [12, N, 64]
    kT = np.transpose(k0, (0, 2, 1)).reshape(HEADS // 2, 128, N)
    kT = np.ascontiguousarray(kT)
    biasT = np.ascontiguousarray(
        np.transpose(ext_bias[0] * SCALE, (0, 2, 1))).astype(ml_dtypes.bfloat16)
    wo = np.ascontiguousarray(w_out)

    in_maps = []
    for c in range(CORES):
        xc = x[c * BPC:(c + 1) * BPC]                # [BPC, N, DIM]
        xT = np.ascontiguousarray(np.transpose(xc, (0, 2, 1)))  # [BPC, DIM, N]
        in_maps.append({"xT": xT, "wq": w_q, "wv": w_v, "kT": kT,
                        "biasT": biasT, "wo": wo})

    res = run_bass_kernel_spmd(nc, in_maps, core_ids=list(range(CORES)))
    out = np.concatenate([res.results[c]["out"] for c in range(CORES)], axis=0)
    out = out.reshape(B, N, DIM) + b_out
    return out.astype(np.float32)


# revision 17
# speedup vs baseline: 136.8886x; 136.8886x over previous
"""Trainium2 Bass kernel for nn_Attention_66640712565145.

Attention with external K and full [h,n,n] bias:
  qv = x @ w_qv ; q,v = split(qv)
  dots = (q @ ext_k^T + ext_bias) * scale ; attn = softmax(dots)
  out = (attn @ v) @ w_out + b_out

Sharding: data-parallel over batch across 8 cores (2 batches/core), no
collectives. Per core the kernel computes S transposed (S^T[m,n], partition =
keys) so the PV matmul needs no transposes; softmax row-sums come from a ones
column embedded in the V tile layout; normalization is applied late to the
small attention output via batched reciprocal + K=1 broadcast matmuls.

Device dtypes: fp32r (TF32-like, full-speed matmul) for all matmuls, bias
pre-scaled and shipped as bf16 (added into PSUM via an identity matmul), fp32
accumulation everywhere.
"""
import numpy as np
import ml_dtypes

HEADS = 12
DIM_HEAD = 64
SCALE = DIM_HEAD ** -0.5
B, N, DIM = 16, 1024, 768
CORES = 8
BPC = B // CORES          # batches per core
T = BPC * N               # tokens per core
NT = N // 128             # 128-row tiles per sequence
VW = DIM_HEAD + 1         # v block width incl. ones column

_program = None
DEBUG = False


def _build_program():
    import concourse.bacc as bacc
    import concourse.mybir as mybir
    from concourse.tile import TileContext
    from concourse.masks import make_identity

    dt = mybir.dt
    f32, f32r, bf16 = dt.float32, dt.float32r, dt.bfloat16
    EXP = mybir.ActivationFunctionType.Exp

    nc = bacc.Bacc(None, target_bir_lowering=False)

    a_xT = nc.declare_dram_parameter("xT", [BPC, DIM, N], f32r, isOutput=False)
    a_wq = nc.declare_dram_parameter("wq", [DIM, DIM], f32r, isOutput=False)
    a_wv = nc.declare_dram_parameter("wv", [DIM, DIM], f32r, isOutput=False)
    a_kT = nc.declare_dram_parameter("kT", [HEADS // 2, 128, N], f32r, isOutput=False)
    a_bias = nc.declare_dram_parameter("biasT", [HEADS, N, N], bf16, isOutput=False)
    a_wo = nc.declare_dram_parameter("wo", [DIM, DIM], f32r, isOutput=False)
    a_out = nc.declare_dram_parameter("out", [T, DIM], f32, isOutput=True)
    qT_dram = nc.dram_tensor("qT_scratch", [BPC, DIM, N], f32r)

    ET = DIM // 128        # e tiles (6)
    if DEBUG:
        d_q = nc.declare_dram_parameter("dbg_q", [BPC, DIM, N], f32, isOutput=True)
        d_v = nc.declare_dram_parameter("dbg_v", [128, BPC * NT * HEADS * VW], f32, isOutput=True)
        d_a = nc.declare_dram_parameter("dbg_a", [128, ET * T], f32, isOutput=True)

    with TileContext(nc) as tc:
        with tc.tile_pool(name="persist", bufs=1) as pers:
            ident = pers.tile([128, 128], bf16, name="ident")
            make_identity(nc, ident)
            ones_f = pers.tile([128, HEADS], f32, name="ones_f")
            nc.vector.memset(ones_f, 1.0)
            ones_r = pers.tile([128, 64], f32r, name="ones_r")
            of64 = pers.tile([128, 64], f32, name="of64")
            nc.vector.memset(of64, 1.0)
            with nc.allow_low_precision("exact const"):
                nc.vector.tensor_copy(ones_r, of64)

            # long-lived buffers
            v_sb = pers.tile([128, BPC * NT * HEADS * VW], f32r, name="v_sb")
            a_sb = pers.tile([128, ET * T], f32r, name="a_sb")

            kpool = tc.alloc_tile_pool(name="ph2_k", bufs=1)
            kT_sb = kpool.tile([128, (HEADS // 2) * N], f32r, name="kT_sb")
            for hp in range(HEADS // 2):
                nc.sync.dma_start(out=kT_sb[:, hp * N:(hp + 1) * N],
                                  in_=a_kT[hp, :, :])

            # ---------------- phase 1: QV projections ----------------
            with tc.tile_pool(name="ph1_w", bufs=1) as wpool, \
                 tc.tile_pool(name="ph1_x", bufs=1) as xpool, \
                 tc.tile_pool(name="ph1_s", bufs=3) as spool, \
                 tc.tile_pool(name="ph1_p", bufs=2, space="PSUM") as ppool:
                wq_t, wv_t = [], []
                for ct in range(ET):
                    t1 = wpool.tile([128, DIM], f32r, name=f"wq{ct}")
                    nc.sync.dma_start(out=t1, in_=a_wq[ct * 128:(ct + 1) * 128, :])
                    wq_t.append(t1)
                    t2 = wpool.tile([128, DIM], f32r, name=f"wv{ct}")
                    nc.sync.dma_start(out=t2, in_=a_wv[ct * 128:(ct + 1) * 128, :])
                    wv_t.append(t2)
                for b in range(BPC):
                    xT_t = []
                    for ct in range(ET):
                        t3 = xpool.tile([128, N], f32r, name=f"xT{ct}")
                        nc.sync.dma_start(out=t3, in_=a_xT[b, ct * 128:(ct + 1) * 128, :])
                        xT_t.append(t3)
                    # q^T: psum[e1, n] = sum_c wq[c, e1].T @ xT[c, n]
                    for et in range(ET):
                        pq = ppool.tile([128, N], f32, name="pq")
                        for ct in range(ET):
                            for nch in range(2):
                                nsl = slice(nch * 512, (nch + 1) * 512)
                                nc.tensor.matmul(
                                    pq[:, nsl],
                                    wq_t[ct][:, et * 128:(et + 1) * 128],
                                    xT_t[ct][:, nsl],
                                    start=(ct == 0), stop=(ct == ET - 1))
                        sq = spool.tile([128, N], f32r, name="sq")
                        with nc.allow_low_precision("fp32r matmul feed"):
                            nc.vector.tensor_copy(sq, pq)
                        nc.sync.dma_start(out=qT_dram[b, et * 128:(et + 1) * 128, :],
                                          in_=sq)
                    # v: psum[t1, e] = sum_c xT[c, t1].T @ wv[c, e]
                    for tt in range(NT):
                        pv = ppool.tile([128, DIM], f32, name="pv")
                        for ct in range(ET):
                            for osl in (slice(0, 512), slice(512, DIM)):
                                nc.tensor.matmul(
                                    pv[:, osl],
                                    xT_t[ct][:, tt * 128:(tt + 1) * 128],
                                    wv_t[ct][:, osl],
                                    start=(ct == 0), stop=(ct == ET - 1))
                        base = (b * NT + tt) * HEADS * VW
                        dst = v_sb[:, base:base + HEADS * VW]
                        dst3 = dst.rearrange("p (h w) -> p h w", w=VW)
                        with nc.allow_low_precision("fp32r matmul feed"):
                            nc.vector.tensor_copy(
                                dst3[:, :, 0:DIM_HEAD],
                                pv.rearrange("p (h w) -> p h w", w=DIM_HEAD))
                            nc.vector.tensor_copy(
                                dst3[:, :, DIM_HEAD:VW].rearrange("p h w -> p (h w)"),
                                ones_f)

            # ---------------- phase 2: attention sweeps ----------------
            with tc.tile_pool(name="ph2_bias", bufs=3) as bpool, \
                 tc.tile_pool(name="ph2_l", bufs=2) as lpool, \
                 tc.tile_pool(name="ph2_q", bufs=2) as qpool, \
                 tc.tile_pool(name="ph2_pt", bufs=3) as ptpool, \
                 tc.tile_pool(name="ph2_s", bufs=2, space="PSUM") as ps_s, \
                 tc.tile_pool(name="ph2_o", bufs=1, space="PSUM") as ps_o:
                for hp in range(HEADS // 2):
                    l_sb = lpool.tile([128, N], f32r, name="l_sb")
                    bias_t = []
                    for h2 in range(2):
                        h = 2 * hp + h2
                        tb = bpool.tile([128, NT * N], bf16, name="biasbuf")
                        nc.gpsimd.dma_start(
                            out=tb.rearrange("p (t n) -> p t n", n=N),
                            in_=a_bias[h, :, :].rearrange("(t p) n -> p t n", p=128))
                        bias_t.append(tb)
                    for b in range(BPC):
                        tq = qpool.tile([128, N], f32r, name="tq")
                        nc.sync.dma_start(out=tq,
                                          in_=qT_dram[b, hp * 128:(hp + 1) * 128, :])
                        po = [ps_o.tile([VW, N], f32, name=f"po{h2}")
                              for h2 in range(2)]
                        for mt in range(NT):
                            pss = [ps_s.tile([128, N], f32, name="ps")
                                   for _ in range(2)]
                            for h2 in range(2):
                                r0 = h2 * 64
                                for nch in range(2):
                                    nsl = slice(nch * 512, (nch + 1) * 512)
                                    nc.tensor.matmul(
                                        pss[h2][:, nsl],
                                        kT_sb[r0:r0 + 64,
                                              hp * N + mt * 128:hp * N + (mt + 1) * 128],
                                        tq[r0:r0 + 64, nsl],
                                        start=True, stop=False,
                                        tile_position=(r0, 0))
                                for nch in range(2):
                                    nsl = slice(nch * 512, (nch + 1) * 512)
                                    nc.tensor.matmul(
                                        pss[h2][:, nsl], ident,
                                        bias_t[h2][:, mt * N:(mt + 1) * N][:, nsl],
                                        start=False, stop=True)
                            for h2 in range(2):
                                h = 2 * hp + h2
                                pt = ptpool.tile([128, N], f32r, name="pt")
                                nc.scalar.activation(pt, pss[h2], EXP)
                                vbase = (b * NT + mt) * HEADS * VW + h * VW
                                vsl = v_sb[:, vbase:vbase + VW]
                                for nch in range(2):
                                    nsl = slice(nch * 512, (nch + 1) * 512)
                                    nc.tensor.matmul(
                                        po[h2][:, nsl], vsl, pt[:, nsl],
                                        start=(mt == 0), stop=(mt == NT - 1))
                        for h2 in range(2):
                            u = hp * 4 + b * 2 + h2          # unit index 0..23
                            with nc.allow_low_precision("fp32r matmul feed"):
                                nc.vector.tensor_copy(
                                    a_sb[h2 * 64:(h2 + 1) * 64,
                                         hp * T + b * N: hp * T + (b + 1) * N],
                                    po[h2][0:DIM_HEAD, :])
                                nc.vector.tensor_copy(
                                    l_sb[(b * 2 + h2) * 32:(b * 2 + h2) * 32 + 1, :],
                                    po[h2][DIM_HEAD:VW, :])
                    # normalization for this head pair (4 units); deprioritized
                    # so it fills PE gaps instead of blocking the next head pair
                    with tc.high_priority(offset=-400):
                        lblk = l_sb
                        with nc.allow_low_precision("softmax reciprocal"):
                            nc.vector.reciprocal(lblk, lblk)
                        for b in range(BPC):
                            for h2 in range(2):
                                row = (b * 2 + h2) * 32
                                pb = ps_s.tile([128, N], f32, name="ps")
                                for nch in range(2):
                                    nsl = slice(nch * 512, (nch + 1) * 512)
                                    nc.tensor.matmul(
                                        pb[0:64, nsl], ones_r[row:row + 1, :],
                                        lblk[row:row + 1, nsl],
                                        start=True, stop=True, tile_position=(row, 0))
                                asl = a_sb[h2 * 64:(h2 + 1) * 64,
                                           hp * T + b * N: hp * T + (b + 1) * N]
                                with nc.allow_low_precision("normalize"):
                                    nc.vector.tensor_mul(asl, asl, pb[0:64, :])

            kpool.release()
            # ---------------- phase 3: normalize + output projection ----
            with tc.tile_pool(name="ph3_w", bufs=1) as wopool, \
                 tc.tile_pool(name="ph3_s", bufs=3) as s3pool, \
                 tc.tile_pool(name="ph3_p", bufs=2, space="PSUM") as p3pool, \
                 tc.tile_pool(name="ph3_pb", bufs=2, space="PSUM") as pbpool:
                wo_t = []
                for et in range(ET):
                    t4 = wopool.tile([128, DIM], f32r, name=f"wo{et}")
                    nc.sync.dma_start(out=t4, in_=a_wo[et * 128:(et + 1) * 128, :])
                    wo_t.append(t4)
                for tt in range(T // 128):
                    pp = p3pool.tile([128, DIM], f32, name="pp")
                    for et in range(ET):
                        for osl in (slice(0, 512), slice(512, DIM)):
                            nc.tensor.matmul(
                                pp[:, osl],
                                a_sb[:, et * T + tt * 128: et * T + (tt + 1) * 128],
                                wo_t[et][:, osl],
                                start=(et == 0), stop=(et == ET - 1))
                    so = s3pool.tile([128, DIM], f32, name="so")
                    nc.vector.tensor_copy(so, pp)
                    nc.sync.dma_start(out=a_out[tt * 128:(tt + 1) * 128, :], in_=so)
                if DEBUG:
                    nc.sync.dma_start(out=d_q[:, :, :], in_=qT_dram[:, :, :].bitcast(f32))
                    nc.sync.dma_start(out=d_v[:, :], in_=v_sb.bitcast(f32))
                    nc.sync.dma_start(out=d_a[:, :], in_=a_sb.bitcast(f32))

    nc.finalize()
    return nc


def _get_program():
    global _program
    if _program is None:
        _program = _build_program()
    return _program


def kernel(x, w_qv, ext_k, ext_bias, w_out, b_out):
    from concourse.bass_utils import run_bass_kernel_spmd

    nc = _get_program()

    x = np.asarray(x, dtype=np.float32)
    w_qv = np.asarray(w_qv, dtype=np.float32)
    ext_k = np.asarray(ext_k, dtype=np.float32)
    ext_bias = np.asarray(ext_bias, dtype=np.float32)
    w_out = np.asarray(w_out, dtype=np.float32)
    b_out = np.asarray(b_out, dtype=np.float32)

    w_q = np.ascontiguousarray(w_qv[:, :DIM] * SCALE)
    w_v = np.ascontiguousarray(w_qv[:, DIM:])
    # kT packed head pairs: [6, 128, N]; rows 0:64 head 2hp, 64:128 head 2hp+1
    k0 = ext_k[0]                                    # [12, N, 64]
    kT = np.transpose(k0, (0, 2, 1)).reshape(HEADS // 2, 128, N)
    kT = np.ascontiguousarray(kT)
    biasT = np.ascontiguousarray(
        np.transpose(ext_bias[0] * SCALE, (0, 2, 1))).astype(ml_dtypes.bfloat16)
    wo = np.ascontiguousarray(w_out)

    in_maps = []
    for c in range(CORES):
        xc = x[c * BPC:(c + 1) * BPC]                # [BPC, N, DIM]
        xT = np.ascontiguousarray(np.transpose(xc, (0, 2, 1)))  # [BPC, DIM, N]
        in_maps.append({"xT": xT, "wq": w_q, "wv": w_v, "kT": kT,
                        "biasT": biasT, "wo": wo})

    res = run_bass_kernel_spmd(nc, in_maps, core_ids=list(range(CORES)))
    out = np.concatenate([res.results[c]["out"] for c in range(CORES)], axis=0)
    out = out.reshape(B, N, DIM) + b_out
    return out.astype(np.float32)
